# revision 16
# baseline (speedup 1.0000x reference)
"""Trainium2 Bass kernel for additive attention (nn_AdditiveAttention).

Reference computation (per batch b):
    q_proj = query @ W1_w.T + W1_b                      # [D]
    v_proj = values @ W2_w.T + W2_b                     # [T, D]
    scores = tanh(q_proj + v_proj) @ v                  # [T]
    weights = softmax(scores)                           # [T]
    out    = weights @ values                           # [E]

Sharding: data-parallel over batch B=32 across 8 NeuronCores (4 batches/core).

The kernel streams `values` from HBM in TWO bf16 layouts prepared on the
host — t-major (partition = t mod 128) for the softmax-numerator matmuls
and e-major (partition = e mod 128) for the v_proj matmuls — so the PE
never spends cycles transposing value tiles (two bf16 streams cost the
same HBM traffic as a single f32 stream, and PE transposes would be ~30%
of tensor-engine time). Tiles are 2048 timesteps (a "quarter"); softmax
spans the full T=8192 row of a batch (one on-chip max/denominator per
batch), and per-batch [num(512), den, M] rows are combined on the host.

Device pipeline per super (512 timesteps):
  - PE v_proj psum [128d, 512t] = W2ed.T @ VT (accum over E)
  - ACT tanh with fused per-partition bias (q_proj + b1 + b2)
  - PE scores psum [1, 512t] = v.T @ tanh (accum over 2 d-chunks)
  v_proj runs one super ahead of scores so the PE always has independent
  queued work while scores wait on tanh latency; the previous batch's
  numerator matmuls fill the score-matmul gaps (4 per super).
Per batch tail (emitted at the START of the next batch, so the whole
chain overlaps that batch's matmul stream):
  - PE transposes score rows -> columns [128, 64] (f32r identity matmul)
  - DVE free-max + GPSIMD partition all-reduce -> batch max M
  - ACT exp(s - M) -> weight cols [128, 64]; DVE row-sum + GPSIMD -> den
  - PE numerator: psum f32 [1, 512e] += w_col.T @ values_tile (64 matmuls)

bf16 operands give ~6e-3 relative error (vs the 2e-2 gate).
"""

import os
import sys
import time

import numpy as np

for _p in ("/opt/trn_rl_repo",):
    if _p not in sys.path and os.path.isdir(_p):
        sys.path.insert(0, _p)

# Problem shapes (hardcoded per contract)
B, T, E, D = 32, 8192, 512, 256
N_CORES = 8
B_LOC = B // N_CORES          # 4 batches per core
P = 128
TSUP = 512                    # timesteps per super tile
JSUB = TSUP // P              # 4 basic 128-t subtiles per super
TQ = 2048                     # timesteps per DMA quarter-tile
NQ = T // TQ                  # 4 quarters per batch
SUP_B = T // TSUP             # 16 supers per batch
NBQ = TQ // P                 # 16 basic tiles per quarter
NB_B = T // P                 # 64 basic tiles (numerator cols) per batch
EC = E // P                   # 4 e-chunks
DC = D // P                   # 2 d-chunks
NSJ = T // P                  # 64 basic t-tiles per batch
OUTW = E + 2                  # num[512], den, M

LAST_RESULT = None            # BassKernelResults of the most recent run


def build_bass(t_loc=T, b_loc=B_LOC, repeat=1, loop_n=1):
    """Build the Bass module (same SPMD program for every core)."""
    import concourse.bacc as bacc
    import concourse.tile as tile
    from concourse import mybir

    f32 = mybir.dt.float32
    dtm = mybir.dt.bfloat16

    assert t_loc == T

    nc = bacc.Bacc("TRN2", target_bir_lowering=False, debug=False,
                   num_devices=N_CORES)
    vals_tm = nc.dram_tensor("vals_tm", [b_loc, P, t_loc // P, E], dtm,
                             kind="ExternalInput").ap()
    vals_et = nc.dram_tensor("vals_et", [b_loc, E, t_loc], dtm,
                             kind="ExternalInput").ap()
    w2ed_d = nc.dram_tensor("w2ed", [E, D], dtm, kind="ExternalInput").ap()
    cb_d = nc.dram_tensor("cb", [D, b_loc], f32, kind="ExternalInput").ap()
    v_d = nc.dram_tensor("vcol", [D, 1], dtm, kind="ExternalInput").ap()
    id32_d = nc.dram_tensor("ident32", [P, P], mybir.dt.float32r,
                            kind="ExternalInput").ap()
    outp = nc.dram_tensor("out_parts", [b_loc, 1, OUTW], f32,
                          kind="ExternalOutput").ap()

    with tile.TileContext(nc) as tc:
        _emit(tc, vals_tm, vals_et, w2ed_d, cb_d, v_d, id32_d, outp, b_loc,
              repeat, loop_n, dtm)
    nc.compile()
    return nc


def _emit(tc, vals_tm, vals_et, w2ed_d, cb_d, v_d, id32_d, outp, b_loc,
          repeat, loop_n, dtm):
    from contextlib import ExitStack

    from concourse import bass_isa, mybir

    f32 = mybir.dt.float32
    f32r = mybir.dt.float32r
    Tanh = mybir.ActivationFunctionType.Tanh
    Exp = mybir.ActivationFunctionType.Exp
    X = mybir.AxisListType.X

    nc = tc.nc

    with ExitStack() as ctx:
        consts = ctx.enter_context(tc.tile_pool(name="consts", bufs=1))
        # vg quarters of batch b live until its numerator completes
        # (which runs during batch b+1) => 4 resident + 4 streaming.
        vpool = ctx.enter_context(tc.tile_pool(name="vpool", bufs=8))
        vtpool = ctx.enter_context(tc.tile_pool(name="vtpool", bufs=3))
        thpool = ctx.enter_context(tc.tile_pool(name="thpool", bufs=4))
        # 4 quarter srow tiles per batch, alive until the softmax at the
        # start of the next batch => 4 resident + 4 filling.
        rowpool = ctx.enter_context(tc.tile_pool(name="rowpool", bufs=8))
        spool = ctx.enter_context(tc.tile_pool(name="spool", bufs=3))
        redpool = ctx.enter_context(tc.tile_pool(name="redpool", bufs=4))
        opool = ctx.enter_context(tc.tile_pool(name="opool", bufs=2))
        # PSUM budget (8 banks): psv 4 + scrow 1 + s4 2 + num 1
        ps_vp = ctx.enter_context(
            tc.tile_pool(name="ps_vp", bufs=4, space="PSUM"))
        ps_sm = ctx.enter_context(
            tc.tile_pool(name="ps_sm", bufs=1, space="PSUM"))
        ps_nm = ctx.enter_context(
            tc.tile_pool(name="ps_nm", bufs=1, space="PSUM"))

        # GPSIMD ucode library providing partition_all_reduce
        from concourse import library_config
        nc.gpsimd.load_library(library_config.mlp)

        # --- constants ---
        w2_sb = consts.tile([P, EC, D], dtm)
        nc.sync.dma_start(w2_sb, w2ed_d.rearrange("(c p) d -> p c d", p=P))
        cb_sb = consts.tile([P, DC, b_loc], f32)
        nc.sync.dma_start(cb_sb, cb_d.rearrange("(c p) b -> p c b", p=P))
        v_sb = consts.tile([P, DC, 1], dtm)
        nc.sync.dma_start(v_sb, v_d.rearrange("(c p) x -> p c x", p=P))
        id32_sb = consts.tile([P, P], f32r)
        nc.sync.dma_start(id32_sb, id32_d)

        num_state = {}

        def emit_num_mms(p, k0, count):
            """Numerator matmuls k0..k0+count for batch p (64 total)."""
            vgs, wg, b, m_all, den_all, rep = p
            key = (rep, b)
            if key not in num_state:
                num_state[key] = ps_nm.tile([1, E], f32, tag="num",
                                            name=f"psn_{rep}_{b}")
            psn = num_state[key]
            for k in range(k0, k0 + count):
                tcn, s = divmod(k, SUP_B)
                sj_g = s * JSUB + tcn
                nc.tensor.matmul(
                    psn,
                    lhsT=wg[:, k:k + 1],
                    rhs=vgs[sj_g // NBQ][:, sj_g % NBQ, :],
                    start=(k == 0), stop=(k == NB_B - 1))

        def emit_num_tail(p):
            vgs, wg, b, m_all, den_all, rep = p
            psn = num_state.pop((rep, b))
            osb = opool.tile([1, 1, OUTW], f32, tag="osb",
                             name=f"osb_{rep}_{b}")
            nc.scalar.copy(osb[:, 0, 0:E], psn)
            nc.vector.tensor_copy(osb[:, 0, E:E + 1], den_all[0:1, :])
            nc.vector.tensor_copy(osb[:, 0, E + 1:E + 2], m_all[0:1, :])
            nc.sync.dma_start(outp[b], osb)

        def emit_softmax(ps, rep):
            """Score rows -> columns, then softmax pieces (max, exp, den).

            Emitted at the START of the following batch so the PE
            transposes slot in right after that batch's first v_proj
            (their srow input is long since ready) and the
            DVE/ACT/GPSIMD softmax chain overlaps the batch's matmul
            stream instead of sitting on the critical path."""
            srows, vgs, b = ps
            # Full 128x128 transposes per quarter srow tile; its 4 score
            # rows sit at partitions {0,32,64,96} (engine writes must be
            # 32-partition aligned), so columns {0,32,64,96} of each
            # transposed block hold the per-basic score columns.
            sg = spool.tile([P, NB_B], f32, tag="sg",
                            name=f"sg_{rep}_{b}")
            for q in range(NQ):
                for tcn in range(JSUB):
                    ps4 = ps_sm.tile([P, P], f32r, tag="s4", bufs=2,
                                     name=f"ps4_{rep}_{b}_{q}_{tcn}")
                    nc.tensor.transpose(
                        ps4,
                        srows[q][:, tcn * P:(tcn + 1) * P],
                        id32_sb)
                    # sg col (tcn*16 + q*4 + sl) = score of super q*4+sl,
                    # basic tcn
                    nc.vector.tensor_copy(
                        sg[:, tcn * SUP_B + q * JSUB:
                           tcn * SUP_B + (q + 1) * JSUB],
                        ps4.rearrange("p (s x) -> p s x", s=JSUB)
                        [:, :, 0:1])

            m_part = redpool.tile([P, 1], f32, tag="mp",
                                  name=f"mp_{rep}_{b}")
            nc.vector.reduce_max(m_part, sg, axis=X)
            m_all = redpool.tile([P, 1], f32, tag="ma",
                                 name=f"ma_{rep}_{b}")
            nc.gpsimd.partition_all_reduce(
                m_all, m_part, channels=P,
                reduce_op=bass_isa.ReduceOp.max)
            negm = redpool.tile([P, 1], f32, tag="nm",
                                name=f"nm_{rep}_{b}")
            nc.vector.tensor_scalar_mul(negm, m_all, -1.0)
            wg = spool.tile([P, NB_B], dtm, tag="wg",
                            name=f"wg_{rep}_{b}")
            nc.scalar.activation(wg, sg, Exp, bias=negm)
            wsum = redpool.tile([P, 1], f32, tag="ws",
                                name=f"ws_{rep}_{b}")
            nc.vector.reduce_sum(wsum, wg, axis=X)
            den_all = redpool.tile([P, 1], f32, tag="da",
                                   name=f"da_{rep}_{b}")
            nc.gpsimd.partition_all_reduce(
                den_all, wsum, channels=P,
                reduce_op=bass_isa.ReduceOp.add)
            return (vgs, wg, b, m_all, den_all, rep)

        def body(rep):
          pending_num = None
          num_done = [0]
          pending_soft = None

          def emit_num(n):
            if pending_num is None:
                return
            n = min(n, NB_B - num_done[0])
            if n > 0:
                emit_num_mms(pending_num, num_done[0], n)
                num_done[0] += n

          for b in range(b_loc):
            # One DMA pair per 2048-t quarter, interleaved so the e-major
            # tile (needed first) always precedes its t-major sibling on
            # the queue.
            vtgs = []
            vgs = []
            for q in range(NQ):
                vtg = vtpool.tile([P, EC, TQ], dtm, tag="vtg",
                                  name=f"vtg_{rep}_{b}_{q}")
                nc.sync.dma_start(
                    vtg,
                    vals_et[b, :, q * TQ:(q + 1) * TQ].rearrange(
                        "(c p) t -> p c t", p=P))
                vtgs.append(vtg)
                vg = vpool.tile([P, NBQ, TSUP], dtm, tag="vg",
                                name=f"vg_{rep}_{b}_{q}")
                nc.sync.dma_start(
                    vg, vals_tm[b, :, q * NBQ:(q + 1) * NBQ, :])
                vgs.append(vg)

            srows = [rowpool.tile([P, TSUP], f32r, tag="srow",
                                  name=f"srow_{rep}_{b}_{q}")
                     for q in range(NQ)]

            def emit_vproj(s):
                """8 PE matmuls + 2 ACT tanh for super s -> th tiles."""
                vtg = vtgs[s // JSUB]
                t0 = (s % JSUB) * TSUP
                ths = []
                for dc in range(DC):
                    psv = ps_vp.tile([P, TSUP], f32, tag="psv",
                                     name=f"psv_{rep}_{b}_{s}_{dc}")
                    for c in range(EC):
                        nc.tensor.matmul(
                            psv,
                            lhsT=w2_sb[:, c, dc * P:(dc + 1) * P],
                            rhs=vtg[:, c, t0:t0 + TSUP],
                            start=(c == 0), stop=(c == EC - 1))
                    th = thpool.tile([P, TSUP], dtm, tag="th",
                                     name=f"th_{rep}_{b}_{s}_{dc}")
                    nc.scalar.activation(th, psv, Tanh,
                                         bias=cb_sb[:, dc, b:b + 1])
                    ths.append(th)
                return ths

            # Software pipeline: v_proj one super ahead of scores so the
            # PE has independent work while scores wait on tanh; the
            # previous batch's numerator matmuls fill the score gaps.
            ths_q = [emit_vproj(0)]
            if pending_soft is not None:
                pending_num = emit_softmax(pending_soft, rep)
                num_done[0] = 0
                pending_soft = None
            for s in range(SUP_B):
                ths = ths_q[s]
                if s + 1 < SUP_B:
                    ths_q.append(emit_vproj(s + 1))
                else:
                    emit_num(8)
                pss = ps_sm.tile([1, TSUP], f32, tag="scrow",
                                 name=f"pss_{rep}_{b}_{s}")
                nc.tensor.matmul(pss, lhsT=v_sb[:, 0, :], rhs=ths[0],
                                 start=True, stop=False)
                if s >= 1:
                    # wg of the previous batch is ready ~4us into this
                    # batch; starting the fillers at s=1 keeps the
                    # in-order PE queue from stalling on it at s=0.
                    emit_num(2)
                nc.tensor.matmul(pss, lhsT=v_sb[:, 1, :], rhs=ths[1],
                                 start=False, stop=True)
                if s >= 1:
                    emit_num(2)
                nc.vector.tensor_copy(
                    srows[s // JSUB][(s % JSUB) * 32:(s % JSUB) * 32 + 1,
                                     :], pss)

            if pending_num is not None:
                emit_num(NB_B)
                emit_num_tail(pending_num)
                pending_num = None

            pending_soft = (srows, vgs, b)

          pending_num = emit_softmax(pending_soft, rep)
          num_done[0] = 0
          emit_num(NB_B)
          emit_num_tail(pending_num)

        if loop_n > 1:
            with tc.For_i(0, loop_n, 1):
                body(0)
        else:
            for rep in range(repeat):
                body(rep)


def host_prepare(values, query, v, W1_w, W1_b, W2_w, W2_b, b_loc=B_LOC,
                 n_cores=N_CORES):
    """Precompute host-side tensors and build per-core input maps."""
    import ml_dtypes

    npm = ml_dtypes.bfloat16

    c = (query.astype(np.float32) @ W1_w.T.astype(np.float32)
         + W1_b + W2_b).astype(np.float32)          # [B, D]
    values = np.asarray(values)
    # t-major: [B, 128p, 64sj, 512e] with t = sj*128 + p
    vals_tm = np.ascontiguousarray(
        values.reshape(B, NSJ, P, E).transpose(0, 2, 1, 3).astype(npm))
    # e-major: [B, 512e, 8192t]
    vals_et = np.ascontiguousarray(values.transpose(0, 2, 1).astype(npm))
    w2ed = np.ascontiguousarray(np.asarray(W2_w).T.astype(npm))  # [E, D]
    vcol = np.ascontiguousarray(np.asarray(v).reshape(D, 1).astype(npm))
    ident32 = np.eye(P, dtype=np.float32)
    in_maps = []
    for k in range(n_cores):
        bsl = slice(k * b_loc, (k + 1) * b_loc)
        in_maps.append({
            "vals_tm": np.ascontiguousarray(vals_tm[bsl]),
            "vals_et": np.ascontiguousarray(vals_et[bsl]),
            "w2ed": w2ed,
            "cb": np.ascontiguousarray(c[bsl].T),    # [D, b_loc]
            "vcol": vcol,
            "ident32": ident32,
        })
    return in_maps


def host_combine(results, b_loc=B_LOC, n_cores=N_CORES):
    """Combine per-(batch, group) partial softmax numerators/denominators."""
    out = np.zeros((n_cores * b_loc, E), np.float32)
    for k in range(n_cores):
        parts = np.asarray(results[k]["out_parts"])  # [b_loc, n_groups, 514]
        num = parts[..., :E].astype(np.float64)
        den = parts[..., E].astype(np.float64)
        M = parts[..., E + 1].astype(np.float64)
        Mb = M.max(axis=1, keepdims=True)
        sc = np.exp(M - Mb)                          # [b_loc, n_groups]
        o = (num * sc[..., None]).sum(1) / (den * sc).sum(1)[:, None]
        out[k * b_loc:(k + 1) * b_loc] = o.astype(np.float32)
    return out


_NC_CACHE = None


def kernel(values, query, v, W1_w, W1_b, W2_w, W2_b):
    global _NC_CACHE, LAST_RESULT
    from concourse.bass_utils import run_bass_kernel_spmd

    in_maps = host_prepare(values, query, v, W1_w, W1_b, W2_w, W2_b)
    if _NC_CACHE is None:
        _NC_CACHE = build_bass()
    trace = bool(int(os.environ.get("KERNEL_TRACE", "0")))
    LAST_RESULT = run_bass_kernel_spmd(
        _NC_CACHE, in_maps, list(range(N_CORES)), trace=trace)
    return host_combine(LAST_RESULT.results)


if __name__ == "__main__":
    rng = np.random.default_rng(0)
    inputs = {
        "values": rng.standard_normal((B, T, E), dtype=np.float32),
        "query": rng.standard_normal((B, D), dtype=np.float32),
        "v": rng.random(D, dtype=np.float32),
        "W1_w": rng.standard_normal((D, D), dtype=np.float32) * 0.06,
        "W1_b": rng.standard_normal(D, dtype=np.float32) * 0.06,
        "W2_w": rng.standard_normal((D, E), dtype=np.float32) * 0.04,
        "W2_b": rng.standard_normal(D, dtype=np.float32) * 0.04,
    }
    t0 = time.time()
    out = kernel(**inputs)
    print("kernel done in", time.time() - t0, "s", out.shape, out.dtype)


# revision 17
# speedup vs baseline: 1.0120x; 1.0120x over previous
"""Trainium2 Bass kernel for additive attention (nn_AdditiveAttention).

Reference computation (per batch b):
    q_proj = query @ W1_w.T + W1_b                      # [D]
    v_proj = values @ W2_w.T + W2_b                     # [T, D]
    scores = tanh(q_proj + v_proj) @ v                  # [T]
    weights = softmax(scores)                           # [T]
    out    = weights @ values                           # [E]

Sharding: data-parallel over batch B=32 across 8 NeuronCores (4 batches/core).

The kernel streams `values` from HBM in TWO bf16 layouts prepared on the
host — t-major (partition = t mod 128) for the softmax-numerator matmuls
and e-major (partition = e mod 128) for the v_proj matmuls — so the PE
never spends cycles transposing value tiles (two bf16 streams cost the
same HBM traffic as a single f32 stream, and PE transposes would be ~30%
of tensor-engine time). Tiles are 2048 timesteps (a "quarter"); softmax
spans the full T=8192 row of a batch (one on-chip max/denominator per
batch), and per-batch [num(512), den, M] rows are combined on the host.

Device pipeline per super (512 timesteps):
  - PE v_proj psum [128d, 512t] = W2ed.T @ VT (accum over E)
  - ACT tanh with fused per-partition bias (q_proj + b1 + b2)
  - PE scores psum [1, 512t] = v.T @ tanh (accum over 2 d-chunks)
  v_proj runs one super ahead of scores so the PE always has independent
  queued work while scores wait on tanh latency; the previous batch's
  numerator matmuls fill the score-matmul gaps (4 per super).
Per batch tail (emitted at the START of the next batch, so the whole
chain overlaps that batch's matmul stream):
  - PE transposes score rows -> columns [128, 64] (f32r identity matmul)
  - DVE free-max + GPSIMD partition all-reduce -> batch max M
  - ACT exp(s - M) -> weight cols [128, 64]; DVE row-sum + GPSIMD -> den
  - PE numerator: psum f32 [1, 512e] += w_col.T @ values_tile (64 matmuls)

bf16 operands give ~6e-3 relative error (vs the 2e-2 gate).
"""

import os
import sys
import time

import numpy as np

for _p in ("/opt/trn_rl_repo",):
    if _p not in sys.path and os.path.isdir(_p):
        sys.path.insert(0, _p)

# Problem shapes (hardcoded per contract)
B, T, E, D = 32, 8192, 512, 256
N_CORES = 8
B_LOC = B // N_CORES          # 4 batches per core
P = 128
TSUP = 512                    # timesteps per super tile
JSUB = TSUP // P              # 4 basic 128-t subtiles per super
TQ = 2048                     # timesteps per DMA quarter-tile
NQ = T // TQ                  # 4 quarters per batch
SUP_B = T // TSUP             # 16 supers per batch
NBQ = TQ // P                 # 16 basic tiles per quarter
NB_B = T // P                 # 64 basic tiles (numerator cols) per batch
EC = E // P                   # 4 e-chunks
DC = D // P                   # 2 d-chunks
NSJ = T // P                  # 64 basic t-tiles per batch
OUTW = E + 2                  # num[512], den, M

LAST_RESULT = None            # BassKernelResults of the most recent run


def build_bass(t_loc=T, b_loc=B_LOC, repeat=1, loop_n=1):
    """Build the Bass module (same SPMD program for every core)."""
    import concourse.bacc as bacc
    import concourse.tile as tile
    from concourse import mybir

    f32 = mybir.dt.float32
    dtm = mybir.dt.bfloat16

    assert t_loc == T

    nc = bacc.Bacc("TRN2", target_bir_lowering=False, debug=False,
                   num_devices=N_CORES)
    vals_tm = nc.dram_tensor("vals_tm", [b_loc, P, t_loc // P, E], dtm,
                             kind="ExternalInput").ap()
    vals_et = nc.dram_tensor("vals_et", [b_loc, E, t_loc], dtm,
                             kind="ExternalInput").ap()
    w2ed_d = nc.dram_tensor("w2ed", [E, D], dtm, kind="ExternalInput").ap()
    cb_d = nc.dram_tensor("cb", [D, b_loc], f32, kind="ExternalInput").ap()
    v_d = nc.dram_tensor("vcol", [D, 1], dtm, kind="ExternalInput").ap()
    id32_d = nc.dram_tensor("ident32", [P, P], mybir.dt.float32r,
                            kind="ExternalInput").ap()
    outp = nc.dram_tensor("out_parts", [b_loc, 1, OUTW], f32,
                          kind="ExternalOutput").ap()

    with tile.TileContext(nc) as tc:
        _emit(tc, vals_tm, vals_et, w2ed_d, cb_d, v_d, id32_d, outp, b_loc,
              repeat, loop_n, dtm)
    nc.compile()
    return nc


def _emit(tc, vals_tm, vals_et, w2ed_d, cb_d, v_d, id32_d, outp, b_loc,
          repeat, loop_n, dtm):
    from contextlib import ExitStack

    from concourse import bass_isa, mybir

    f32 = mybir.dt.float32
    f32r = mybir.dt.float32r
    Tanh = mybir.ActivationFunctionType.Tanh
    Exp = mybir.ActivationFunctionType.Exp
    X = mybir.AxisListType.X

    nc = tc.nc

    with ExitStack() as ctx:
        consts = ctx.enter_context(tc.tile_pool(name="consts", bufs=1))
        # vg quarters of batch b live until its numerator completes
        # (which runs during batch b+1) => 4 resident + 4 streaming.
        vpool = ctx.enter_context(tc.tile_pool(name="vpool", bufs=8))
        vtpool = ctx.enter_context(tc.tile_pool(name="vtpool", bufs=3))
        thpool = ctx.enter_context(tc.tile_pool(name="thpool", bufs=4))
        # 4 quarter srow tiles per batch, alive until the softmax at the
        # start of the next batch => 4 resident + 4 filling.
        rowpool = ctx.enter_context(tc.tile_pool(name="rowpool", bufs=8))
        spool = ctx.enter_context(tc.tile_pool(name="spool", bufs=3))
        redpool = ctx.enter_context(tc.tile_pool(name="redpool", bufs=4))
        opool = ctx.enter_context(tc.tile_pool(name="opool", bufs=2))
        # PSUM budget (8 banks): psv 4 + scrow 1 + s4 2 + num 1
        ps_vp = ctx.enter_context(
            tc.tile_pool(name="ps_vp", bufs=4, space="PSUM"))
        ps_sm = ctx.enter_context(
            tc.tile_pool(name="ps_sm", bufs=1, space="PSUM"))
        ps_nm = ctx.enter_context(
            tc.tile_pool(name="ps_nm", bufs=1, space="PSUM"))

        # GPSIMD ucode library providing partition_all_reduce
        from concourse import library_config
        nc.gpsimd.load_library(library_config.mlp)

        # --- constants ---
        w2_sb = consts.tile([P, EC, D], dtm)
        nc.sync.dma_start(w2_sb, w2ed_d.rearrange("(c p) d -> p c d", p=P))
        cb_sb = consts.tile([P, DC, b_loc], f32)
        nc.sync.dma_start(cb_sb, cb_d.rearrange("(c p) b -> p c b", p=P))
        v_sb = consts.tile([P, DC, 1], dtm)
        nc.sync.dma_start(v_sb, v_d.rearrange("(c p) x -> p c x", p=P))
        id32_sb = consts.tile([P, P], f32r)
        nc.sync.dma_start(id32_sb, id32_d)

        num_state = {}

        def emit_num_mms(p, k0, count):
            """Numerator matmuls k0..k0+count for batch p (64 total)."""
            vgs, wg, b, m_all, den_all, rep = p
            key = (rep, b)
            if key not in num_state:
                num_state[key] = ps_nm.tile([1, E], f32, tag="num",
                                            name=f"psn_{rep}_{b}")
            psn = num_state[key]
            for k in range(k0, k0 + count):
                tcn, s = divmod(k, SUP_B)
                sj_g = s * JSUB + tcn
                nc.tensor.matmul(
                    psn,
                    lhsT=wg[:, k:k + 1],
                    rhs=vgs[sj_g // NBQ][:, sj_g % NBQ, :],
                    start=(k == 0), stop=(k == NB_B - 1))

        def emit_num_tail(p):
            vgs, wg, b, m_all, den_all, rep = p
            psn = num_state.pop((rep, b))
            osb = opool.tile([1, 1, OUTW], f32, tag="osb",
                             name=f"osb_{rep}_{b}")
            nc.scalar.copy(osb[:, 0, 0:E], psn)
            nc.vector.tensor_copy(osb[:, 0, E:E + 1], den_all[0:1, :])
            nc.vector.tensor_copy(osb[:, 0, E + 1:E + 2], m_all[0:1, :])
            nc.sync.dma_start(outp[b], osb)

        def emit_softmax(ps, rep):
            """Score rows -> columns, then softmax pieces (max, exp, den).

            Emitted at the START of the following batch so the PE
            transposes slot in right after that batch's first v_proj
            (their srow input is long since ready) and the
            DVE/ACT/GPSIMD softmax chain overlaps the batch's matmul
            stream instead of sitting on the critical path."""
            srows, vgs, b = ps
            # Full 128x128 transposes per quarter srow tile; its 4 score
            # rows sit at partitions {0,32,64,96} (engine writes must be
            # 32-partition aligned), so columns {0,32,64,96} of each
            # transposed block hold the per-basic score columns.
            sg = spool.tile([P, NB_B], f32, tag="sg",
                            name=f"sg_{rep}_{b}")
            for q in range(NQ):
                for tcn in range(JSUB):
                    ps4 = ps_sm.tile([P, P], f32r, tag="s4", bufs=2,
                                     name=f"ps4_{rep}_{b}_{q}_{tcn}")
                    nc.tensor.transpose(
                        ps4,
                        srows[q][:, tcn * P:(tcn + 1) * P],
                        id32_sb)
                    # sg col (tcn*16 + q*4 + sl) = score of super q*4+sl,
                    # basic tcn
                    nc.vector.tensor_copy(
                        sg[:, tcn * SUP_B + q * JSUB:
                           tcn * SUP_B + (q + 1) * JSUB],
                        ps4.rearrange("p (s x) -> p s x", s=JSUB)
                        [:, :, 0:1])

            m_part = redpool.tile([P, 1], f32, tag="mp",
                                  name=f"mp_{rep}_{b}")
            nc.vector.reduce_max(m_part, sg, axis=X)
            m_all = redpool.tile([P, 1], f32, tag="ma",
                                 name=f"ma_{rep}_{b}")
            nc.gpsimd.partition_all_reduce(
                m_all, m_part, channels=P,
                reduce_op=bass_isa.ReduceOp.max)
            negm = redpool.tile([P, 1], f32, tag="nm",
                                name=f"nm_{rep}_{b}")
            nc.vector.tensor_scalar_mul(negm, m_all, -1.0)
            wg = spool.tile([P, NB_B], dtm, tag="wg",
                            name=f"wg_{rep}_{b}")
            nc.scalar.activation(wg, sg, Exp, bias=negm)
            wsum = redpool.tile([P, 1], f32, tag="ws",
                                name=f"ws_{rep}_{b}")
            nc.vector.reduce_sum(wsum, wg, axis=X)
            den_all = redpool.tile([P, 1], f32, tag="da",
                                   name=f"da_{rep}_{b}")
            nc.gpsimd.partition_all_reduce(
                den_all, wsum, channels=P,
                reduce_op=bass_isa.ReduceOp.add)
            return (vgs, wg, b, m_all, den_all, rep)

        def body(rep):
          pending_num = None
          num_done = [0]
          pending_soft = None

          def emit_num(n):
            if pending_num is None:
                return
            n = min(n, NB_B - num_done[0])
            if n > 0:
                emit_num_mms(pending_num, num_done[0], n)
                num_done[0] += n

          for b in range(b_loc):
            # One DMA pair per 2048-t quarter, interleaved so the e-major
            # tile (needed first) always precedes its t-major sibling on
            # the queue.
            vtgs = []
            vgs = []
            for q in range(NQ):
                vtg = vtpool.tile([P, EC, TQ], dtm, tag="vtg",
                                  name=f"vtg_{rep}_{b}_{q}")
                nc.sync.dma_start(
                    vtg,
                    vals_et[b, :, q * TQ:(q + 1) * TQ].rearrange(
                        "(c p) t -> p c t", p=P))
                vtgs.append(vtg)
                vg = vpool.tile([P, NBQ, TSUP], dtm, tag="vg",
                                name=f"vg_{rep}_{b}_{q}")
                nc.sync.dma_start(
                    vg, vals_tm[b, :, q * NBQ:(q + 1) * NBQ, :])
                vgs.append(vg)

            srows = [rowpool.tile([P, TSUP], f32r, tag="srow",
                                  name=f"srow_{rep}_{b}_{q}")
                     for q in range(NQ)]

            def emit_vproj(s):
                """8 PE matmuls + 2 ACT tanh for super s -> th tiles."""
                vtg = vtgs[s // JSUB]
                t0 = (s % JSUB) * TSUP
                ths = []
                for dc in range(DC):
                    psv = ps_vp.tile([P, TSUP], f32, tag="psv",
                                     name=f"psv_{rep}_{b}_{s}_{dc}")
                    for c in range(EC):
                        nc.tensor.matmul(
                            psv,
                            lhsT=w2_sb[:, c, dc * P:(dc + 1) * P],
                            rhs=vtg[:, c, t0:t0 + TSUP],
                            start=(c == 0), stop=(c == EC - 1))
                    th = thpool.tile([P, TSUP], dtm, tag="th",
                                     name=f"th_{rep}_{b}_{s}_{dc}")
                    nc.scalar.activation(th, psv, Tanh,
                                         bias=cb_sb[:, dc, b:b + 1])
                    ths.append(th)
                return ths

            # Software pipeline: v_proj one super ahead of scores so the
            # PE has independent work while scores wait on tanh; the
            # previous batch's numerator matmuls fill the score gaps.
            ths_q = [emit_vproj(0)]
            if pending_soft is not None:
                pending_num = emit_softmax(pending_soft, rep)
                num_done[0] = 0
                pending_soft = None
            for s in range(SUP_B):
                ths = ths_q[s]
                if s + 1 < SUP_B:
                    ths_q.append(emit_vproj(s + 1))
                else:
                    emit_num(8)
                pss = ps_sm.tile([1, TSUP], f32, tag="scrow",
                                 name=f"pss_{rep}_{b}_{s}")
                nc.tensor.matmul(pss, lhsT=v_sb[:, 0, :], rhs=ths[0],
                                 start=True, stop=False)
                if s >= 2:
                    # wg of the previous batch is ready ~7us into this
                    # batch (16 transposes + sg assembly + softmax chain);
                    # starting the fillers at s=2 keeps the in-order PE
                    # queue from stalling on it.
                    emit_num(2)
                nc.tensor.matmul(pss, lhsT=v_sb[:, 1, :], rhs=ths[1],
                                 start=False, stop=True)
                if s >= 2:
                    emit_num(2)
                nc.vector.tensor_copy(
                    srows[s // JSUB][(s % JSUB) * 32:(s % JSUB) * 32 + 1,
                                     :], pss)

            if pending_num is not None:
                emit_num(NB_B)
                emit_num_tail(pending_num)
                pending_num = None

            pending_soft = (srows, vgs, b)

          pending_num = emit_softmax(pending_soft, rep)
          num_done[0] = 0
          emit_num(NB_B)
          emit_num_tail(pending_num)

        if loop_n > 1:
            with tc.For_i(0, loop_n, 1):
                body(0)
        else:
            for rep in range(repeat):
                body(rep)


def host_prepare(values, query, v, W1_w, W1_b, W2_w, W2_b, b_loc=B_LOC,
                 n_cores=N_CORES):
    """Precompute host-side tensors and build per-core input maps."""
    import ml_dtypes

    npm = ml_dtypes.bfloat16

    c = (query.astype(np.float32) @ W1_w.T.astype(np.float32)
         + W1_b + W2_b).astype(np.float32)          # [B, D]
    values = np.asarray(values)
    # t-major: [B, 128p, 64sj, 512e] with t = sj*128 + p
    vals_tm = np.ascontiguousarray(
        values.reshape(B, NSJ, P, E).transpose(0, 2, 1, 3).astype(npm))
    # e-major: [B, 512e, 8192t]
    vals_et = np.ascontiguousarray(values.transpose(0, 2, 1).astype(npm))
    w2ed = np.ascontiguousarray(np.asarray(W2_w).T.astype(npm))  # [E, D]
    vcol = np.ascontiguousarray(np.asarray(v).reshape(D, 1).astype(npm))
    ident32 = np.eye(P, dtype=np.float32)
    in_maps = []
    for k in range(n_cores):
        bsl = slice(k * b_loc, (k + 1) * b_loc)
        in_maps.append({
            "vals_tm": np.ascontiguousarray(vals_tm[bsl]),
            "vals_et": np.ascontiguousarray(vals_et[bsl]),
            "w2ed": w2ed,
            "cb": np.ascontiguousarray(c[bsl].T),    # [D, b_loc]
            "vcol": vcol,
            "ident32": ident32,
        })
    return in_maps


def host_combine(results, b_loc=B_LOC, n_cores=N_CORES):
    """Combine per-(batch, group) partial softmax numerators/denominators."""
    out = np.zeros((n_cores * b_loc, E), np.float32)
    for k in range(n_cores):
        parts = np.asarray(results[k]["out_parts"])  # [b_loc, n_groups, 514]
        num = parts[..., :E].astype(np.float64)
        den = parts[..., E].astype(np.float64)
        M = parts[..., E + 1].astype(np.float64)
        Mb = M.max(axis=1, keepdims=True)
        sc = np.exp(M - Mb)                          # [b_loc, n_groups]
        o = (num * sc[..., None]).sum(1) / (den * sc).sum(1)[:, None]
        out[k * b_loc:(k + 1) * b_loc] = o.astype(np.float32)
    return out


_NC_CACHE = None


def kernel(values, query, v, W1_w, W1_b, W2_w, W2_b):
    global _NC_CACHE, LAST_RESULT
    from concourse.bass_utils import run_bass_kernel_spmd

    in_maps = host_prepare(values, query, v, W1_w, W1_b, W2_w, W2_b)
    if _NC_CACHE is None:
        _NC_CACHE = build_bass()
    trace = bool(int(os.environ.get("KERNEL_TRACE", "0")))
    LAST_RESULT = run_bass_kernel_spmd(
        _NC_CACHE, in_maps, list(range(N_CORES)), trace=trace)
    return host_combine(LAST_RESULT.results)


if __name__ == "__main__":
    rng = np.random.default_rng(0)
    inputs = {
        "values": rng.standard_normal((B, T, E), dtype=np.float32),
        "query": rng.standard_normal((B, D), dtype=np.float32),
        "v": rng.random(D, dtype=np.float32),
        "W1_w": rng.standard_normal((D, D), dtype=np.float32) * 0.06,
        "W1_b": rng.standard_normal(D, dtype=np.float32) * 0.06,
        "W2_w": rng.standard_normal((D, E), dtype=np.float32) * 0.04,
        "W2_b": rng.standard_normal(D, dtype=np.float32) * 0.04,
    }
    t0 = time.time()
    out = kernel(**inputs)
    print("kernel done in", time.time() - t0, "s", out.shape, out.dtype)


# revision 18
# speedup vs baseline: 1.0697x; 1.0569x over previous
"""Trainium2 Bass kernel for additive attention (nn_AdditiveAttention).

Reference computation (per batch b):
    q_proj = query @ W1_w.T + W1_b                      # [D]
    v_proj = values @ W2_w.T + W2_b                     # [T, D]
    scores = tanh(q_proj + v_proj) @ v                  # [T]
    weights = softmax(scores)                           # [T]
    out    = weights @ values                           # [E]

Sharding: data-parallel over batch B=32 across 8 NeuronCores (4 batches/core).

The kernel streams `values` from HBM in TWO bf16 layouts prepared on the
host — t-major (partition = t mod 128) for the softmax-numerator matmuls
and e-major (partition = e mod 128) for the v_proj matmuls — so the PE
never spends cycles transposing value tiles (two bf16 streams cost the
same HBM traffic as the old single f32 stream, and the old PE transposes
were ~30%% of tensor-engine time). Softmax is computed blockwise per group
of 2048 timesteps (group max + unnormalized numerator/denominator);
groups are combined on the host.

Device pipeline per group (2048 timesteps, 4 supers of 512):
  - group DMAs: e-major [128e, 4c, 2048t] and t-major [128t, 16sj, 512e]
  - per super: PE v_proj psum [128d, 512t] = W2ed.T @ VT (accum over E),
    ACT tanh with fused per-partition bias (q_proj + b1 + b2),
    PE scores psum [1, 512t] = v.T @ tanh (accum over 2 d-chunks)
  - v_proj runs one super ahead of scores so the PE always has
    independent queued work while scores wait on tanh latency; the
    previous group's numerator matmuls fill the remaining gaps.
Per group tail:
  - PE transposes score rows -> columns [128, 16] (f32r identity matmul)
  - DVE free-max + GPSIMD partition all-reduce -> group max M
  - ACT exp(s - M) -> weight cols [128, 16]; DVE row-sum + GPSIMD -> den
  - PE numerator: psum f32 [1, 512e] += w_col.T @ values_tile (16 matmuls)
  - per (batch, group) output row: [num(512), den, M] -> host combine

bf16 operands give ~4e-3 relative error (vs the 2e-2 gate).
"""

import os
import sys
import time

import numpy as np

for _p in ("/opt/trn_rl_repo",):
    if _p not in sys.path and os.path.isdir(_p):
        sys.path.insert(0, _p)

# Problem shapes (hardcoded per contract)
B, T, E, D = 32, 8192, 512, 256
N_CORES = 8
B_LOC = B // N_CORES          # 4 batches per core
P = 128
TSUP = 512                    # timesteps per super tile
JSUB = TSUP // P              # 4 basic 128-t subtiles per super
SUP_PER_GROUP = 4             # supers per softmax group
T_GROUP = TSUP * SUP_PER_GROUP  # 2048
EC = E // P                   # 4 e-chunks
DC = D // P                   # 2 d-chunks
NB = SUP_PER_GROUP * JSUB     # 16 basic tiles (numerator cols) per group
NSJ = T // P                  # 64 basic t-tiles per batch
OUTW = E + 2                  # num[512], den, M

LAST_RESULT = None            # BassKernelResults of the most recent run


def build_bass(t_loc=T, b_loc=B_LOC, repeat=1, loop_n=1):
    """Build the Bass module (same SPMD program for every core)."""
    import concourse.bacc as bacc
    import concourse.tile as tile
    from concourse import mybir

    f32 = mybir.dt.float32
    dtm = mybir.dt.bfloat16

    n_groups = t_loc // T_GROUP
    assert t_loc % T_GROUP == 0

    nc = bacc.Bacc("TRN2", target_bir_lowering=False, debug=False,
                   num_devices=N_CORES)
    vals_tm = nc.dram_tensor("vals_tm", [b_loc, P, t_loc // P, E], dtm,
                             kind="ExternalInput").ap()
    vals_et = nc.dram_tensor("vals_et", [b_loc, E, t_loc], dtm,
                             kind="ExternalInput").ap()
    w2ed_d = nc.dram_tensor("w2ed", [E, D], dtm, kind="ExternalInput").ap()
    cb_d = nc.dram_tensor("cb", [D, b_loc], f32, kind="ExternalInput").ap()
    v_d = nc.dram_tensor("vcol", [D, 1], dtm, kind="ExternalInput").ap()
    id32_d = nc.dram_tensor("ident32", [P, P], mybir.dt.float32r,
                            kind="ExternalInput").ap()
    outp = nc.dram_tensor("out_parts", [b_loc, n_groups, OUTW], f32,
                          kind="ExternalOutput").ap()

    with tile.TileContext(nc) as tc:
        _emit(tc, vals_tm, vals_et, w2ed_d, cb_d, v_d, id32_d, outp, b_loc,
              n_groups, repeat, loop_n, dtm)
    nc.compile()
    return nc


def _emit(tc, vals_tm, vals_et, w2ed_d, cb_d, v_d, id32_d, outp, b_loc,
          n_groups, repeat, loop_n, dtm):
    from contextlib import ExitStack

    from concourse import bass_isa, mybir

    f32 = mybir.dt.float32
    f32r = mybir.dt.float32r
    Tanh = mybir.ActivationFunctionType.Tanh
    Exp = mybir.ActivationFunctionType.Exp
    X = mybir.AxisListType.X

    nc = tc.nc

    with ExitStack() as ctx:
        consts = ctx.enter_context(tc.tile_pool(name="consts", bufs=1))
        vpool = ctx.enter_context(tc.tile_pool(name="vpool", bufs=4))
        vtpool = ctx.enter_context(tc.tile_pool(name="vtpool", bufs=4))
        thpool = ctx.enter_context(tc.tile_pool(name="thpool", bufs=4))
        rowpool = ctx.enter_context(tc.tile_pool(name="rowpool", bufs=2))
        spool = ctx.enter_context(tc.tile_pool(name="spool", bufs=3))
        redpool = ctx.enter_context(tc.tile_pool(name="redpool", bufs=4))
        opool = ctx.enter_context(tc.tile_pool(name="opool", bufs=4))
        # PSUM budget (8 banks): psv 4 + scrow 1 + s4 2 + num 1
        ps_vp = ctx.enter_context(
            tc.tile_pool(name="ps_vp", bufs=4, space="PSUM"))
        ps_sm = ctx.enter_context(
            tc.tile_pool(name="ps_sm", bufs=1, space="PSUM"))
        ps_nm = ctx.enter_context(
            tc.tile_pool(name="ps_nm", bufs=1, space="PSUM"))

        # GPSIMD ucode library providing partition_all_reduce
        from concourse import library_config
        nc.gpsimd.load_library(library_config.mlp)

        # --- constants ---
        w2_sb = consts.tile([P, EC, D], dtm)
        nc.sync.dma_start(w2_sb, w2ed_d.rearrange("(c p) d -> p c d", p=P))
        cb_sb = consts.tile([P, DC, b_loc], f32)
        nc.sync.dma_start(cb_sb, cb_d.rearrange("(c p) b -> p c b", p=P))
        v_sb = consts.tile([P, DC, 1], dtm)
        nc.sync.dma_start(v_sb, v_d.rearrange("(c p) x -> p c x", p=P))
        id32_sb = consts.tile([P, P], f32r)
        nc.sync.dma_start(id32_sb, id32_d)

        num_state = {}

        def emit_numerator_half(p, k0, count):
            vg, wg, b, g, m_all, den_all, rep = p
            key = (rep, b, g)
            if key not in num_state:
                num_state[key] = ps_nm.tile([1, E], f32, tag="num",
                                            name=f"psn_{rep}_{b}_{g}")
            psn = num_state[key]
            for k in range(k0, k0 + count):
                tcn, s = divmod(k, SUP_PER_GROUP)
                col = tcn * SUP_PER_GROUP + s
                nc.tensor.matmul(
                    psn,
                    lhsT=wg[:, col:col + 1],
                    rhs=vg[:, s * JSUB + tcn, :],
                    start=(k == 0), stop=(k == NB - 1))

        osb_state = {}

        def emit_numerator_tail(p):
            vg, wg, b, g, m_all, den_all, rep = p
            psn = num_state.pop((rep, b, g))
            key = (rep, b)
            if key not in osb_state:
                osb_state[key] = opool.tile([1, n_groups, OUTW], f32,
                                            tag="osb", bufs=2,
                                            name=f"osb_{rep}_{b}")
            osb = osb_state[key]
            nc.scalar.copy(osb[:, g, 0:E], psn)
            nc.vector.tensor_copy(osb[:, g, E:E + 1], den_all[0:1, :])
            nc.vector.tensor_copy(osb[:, g, E + 1:E + 2], m_all[0:1, :])
            if g == n_groups - 1:
                nc.sync.dma_start(outp[b], osb_state.pop(key))

        def emit_numerator(p):
            emit_numerator_half(p, 0, NB)
            emit_numerator_tail(p)

        def emit_softmax(ps, rep):
            """Score rows -> columns, then softmax pieces (max, exp, den).

            Emitted at the START of the following group so the PE
            transposes slot in right after that group's first v_proj
            batch (their srow input is long since ready) and the
            DVE/ACT/GPSIMD softmax chain overlaps the group's matmul
            stream instead of sitting on the critical path."""
            srow_g, vg, b, g = ps
            # Full 128x128 transpose; meaningful score rows sit at
            # partitions {0,32,64,96}, so cols {0,32,64,96} of the
            # transposed tile hold the per-basic score columns.
            sg = spool.tile([P, NB], f32, tag="sg",
                            name=f"sg_{rep}_{b}_{g}")
            for tcn in range(JSUB):
                ps4 = ps_sm.tile([P, P], f32r, tag="s4", bufs=2,
                                 name=f"ps4_{rep}_{b}_{g}_{tcn}")
                nc.tensor.transpose(
                    ps4,
                    srow_g[:, tcn * P:(tcn + 1) * P],
                    id32_sb)
                nc.vector.tensor_copy(
                    sg[:, tcn * SUP_PER_GROUP:(tcn + 1) * SUP_PER_GROUP],
                    ps4.rearrange("p (s x) -> p s x", s=SUP_PER_GROUP)
                    [:, :, 0:1])

            m_part = redpool.tile([P, 1], f32, tag="mp",
                                  name=f"mp_{rep}_{b}_{g}")
            nc.vector.reduce_max(m_part, sg, axis=X)
            m_all = redpool.tile([P, 1], f32, tag="ma",
                                 name=f"ma_{rep}_{b}_{g}")
            nc.gpsimd.partition_all_reduce(
                m_all, m_part, channels=P,
                reduce_op=bass_isa.ReduceOp.max)
            negm = redpool.tile([P, 1], f32, tag="nm",
                                name=f"nm_{rep}_{b}_{g}")
            nc.vector.tensor_scalar_mul(negm, m_all, -1.0)
            wg = spool.tile([P, NB], dtm, tag="wg",
                            name=f"wg_{rep}_{b}_{g}")
            nc.scalar.activation(wg, sg, Exp, bias=negm)
            wsum = redpool.tile([P, 1], f32, tag="ws",
                                name=f"ws_{rep}_{b}_{g}")
            nc.vector.reduce_sum(wsum, wg, axis=X)
            den_all = redpool.tile([P, 1], f32, tag="da",
                                   name=f"da_{rep}_{b}_{g}")
            nc.gpsimd.partition_all_reduce(
                den_all, wsum, channels=P,
                reduce_op=bass_isa.ReduceOp.add)
            return (vg, wg, b, g, m_all, den_all, rep)

        def body(rep):
          pending_num = None
          pending_soft = None
          for b in range(b_loc):
            for g in range(n_groups):
                t0g = g * T_GROUP
                # e-major tile for v_proj: [128e, 4c, 2048t]
                vtg = vtpool.tile([P, EC, T_GROUP], dtm, tag="vtg",
                                  name=f"vtg_{rep}_{b}_{g}")
                nc.sync.dma_start(
                    vtg,
                    vals_et[b, :, t0g:t0g + T_GROUP].rearrange(
                        "(c p) t -> p c t", p=P))
                # t-major tile for the numerator: [128t, 16sj, 512e]
                vg = vpool.tile([P, NB, TSUP], dtm, tag="vg",
                                name=f"vg_{rep}_{b}_{g}")
                nc.sync.dma_start(
                    vg, vals_tm[b, :, g * NB:(g + 1) * NB, :])

                srow_g = rowpool.tile([P, TSUP], f32r, tag="srow",
                                      name=f"srow_{rep}_{b}_{g}")

                def emit_vproj(s):
                    """8 PE matmuls + 2 ACT tanh for super s -> th tiles."""
                    ths = []
                    for dc in range(DC):
                        psv = ps_vp.tile([P, TSUP], f32, tag="psv",
                                         name=f"psv_{rep}_{b}_{g}_{s}_{dc}")
                        for c in range(EC):
                            nc.tensor.matmul(
                                psv,
                                lhsT=w2_sb[:, c, dc * P:(dc + 1) * P],
                                rhs=vtg[:, c, s * TSUP:(s + 1) * TSUP],
                                start=(c == 0), stop=(c == EC - 1))
                        th = thpool.tile([P, TSUP], dtm, tag="th",
                                         name=f"th_{rep}_{b}_{g}_{s}_{dc}")
                        nc.scalar.activation(th, psv, Tanh,
                                             bias=cb_sb[:, dc, b:b + 1])
                        ths.append(th)
                    return ths

                # Software pipeline: v_proj one super ahead of scores so
                # the PE has independent work while scores wait on tanh;
                # the previous group's numerator matmuls fill the tail.
                ths_q = [emit_vproj(0)]
                if pending_soft is not None:
                    pending_num = emit_softmax(pending_soft, rep)
                    pending_soft = None
                for s in range(SUP_PER_GROUP):
                    ths = ths_q[s]
                    if s + 1 < SUP_PER_GROUP:
                        ths_q.append(emit_vproj(s + 1))
                    elif pending_num is not None:
                        emit_numerator_half(pending_num, 0, NB // 2)
                    pss = ps_sm.tile([1, TSUP], f32, tag="scrow",
                                     name=f"pss_{rep}_{b}_{g}_{s}")
                    nc.tensor.matmul(pss, lhsT=v_sb[:, 0, :], rhs=ths[0],
                                     start=True, stop=False)
                    if s == SUP_PER_GROUP - 1 and pending_num is not None:
                        emit_numerator_half(pending_num, NB // 2, NB // 4)
                    nc.tensor.matmul(pss, lhsT=v_sb[:, 1, :], rhs=ths[1],
                                     start=False, stop=True)
                    if s == SUP_PER_GROUP - 1 and pending_num is not None:
                        emit_numerator_half(pending_num, 3 * NB // 4, NB // 4)
                        emit_numerator_tail(pending_num)
                        pending_num = None
                    nc.vector.tensor_copy(
                        srow_g[s * 32:s * 32 + 1, :], pss)

                if pending_num is not None:
                    emit_numerator(pending_num)
                    pending_num = None

                pending_soft = (srow_g, vg, b, g)
          pending_num = emit_softmax(pending_soft, rep)
          emit_numerator(pending_num)

        if loop_n > 1:
            with tc.For_i(0, loop_n, 1):
                body(0)
        else:
            for rep in range(repeat):
                body(rep)


def host_prepare(values, query, v, W1_w, W1_b, W2_w, W2_b, b_loc=B_LOC,
                 n_cores=N_CORES):
    """Precompute host-side tensors and build per-core input maps."""
    import ml_dtypes

    npm = ml_dtypes.bfloat16

    c = (query.astype(np.float32) @ W1_w.T.astype(np.float32)
         + W1_b + W2_b).astype(np.float32)          # [B, D]
    values = np.asarray(values)
    # t-major: [B, 128p, 64sj, 512e] with t = sj*128 + p
    vals_tm = np.ascontiguousarray(
        values.reshape(B, NSJ, P, E).transpose(0, 2, 1, 3).astype(npm))
    # e-major: [B, 512e, 8192t]
    vals_et = np.ascontiguousarray(values.transpose(0, 2, 1).astype(npm))
    w2ed = np.ascontiguousarray(np.asarray(W2_w).T.astype(npm))  # [E, D]
    vcol = np.ascontiguousarray(np.asarray(v).reshape(D, 1).astype(npm))
    ident32 = np.eye(P, dtype=np.float32)
    in_maps = []
    for k in range(n_cores):
        bsl = slice(k * b_loc, (k + 1) * b_loc)
        in_maps.append({
            "vals_tm": np.ascontiguousarray(vals_tm[bsl]),
            "vals_et": np.ascontiguousarray(vals_et[bsl]),
            "w2ed": w2ed,
            "cb": np.ascontiguousarray(c[bsl].T),    # [D, b_loc]
            "vcol": vcol,
            "ident32": ident32,
        })
    return in_maps


def host_combine(results, b_loc=B_LOC, n_cores=N_CORES):
    """Combine per-(batch, group) partial softmax numerators/denominators."""
    out = np.zeros((n_cores * b_loc, E), np.float32)
    for k in range(n_cores):
        parts = np.asarray(results[k]["out_parts"])  # [b_loc, n_groups, 514]
        num = parts[..., :E].astype(np.float64)
        den = parts[..., E].astype(np.float64)
        M = parts[..., E + 1].astype(np.float64)
        Mb = M.max(axis=1, keepdims=True)
        sc = np.exp(M - Mb)                          # [b_loc, n_groups]
        o = (num * sc[..., None]).sum(1) / (den * sc).sum(1)[:, None]
        out[k * b_loc:(k + 1) * b_loc] = o.astype(np.float32)
    return out


_NC_CACHE = None


def kernel(values, query, v, W1_w, W1_b, W2_w, W2_b):
    global _NC_CACHE, LAST_RESULT
    from concourse.bass_utils import run_bass_kernel_spmd

    in_maps = host_prepare(values, query, v, W1_w, W1_b, W2_w, W2_b)
    if _NC_CACHE is None:
        _NC_CACHE = build_bass()
    trace = bool(int(os.environ.get("KERNEL_TRACE", "0")))
    LAST_RESULT = run_bass_kernel_spmd(
        _NC_CACHE, in_maps, list(range(N_CORES)), trace=trace)
    return host_combine(LAST_RESULT.results)


if __name__ == "__main__":
    rng = np.random.default_rng(0)
    inputs = {
        "values": rng.standard_normal((B, T, E), dtype=np.float32),
        "query": rng.standard_normal((B, D), dtype=np.float32),
        "v": rng.random(D, dtype=np.float32),
        "W1_w": rng.standard_normal((D, D), dtype=np.float32) * 0.06,
        "W1_b": rng.standard_normal(D, dtype=np.float32) * 0.06,
        "W2_w": rng.standard_normal((D, E), dtype=np.float32) * 0.04,
        "W2_b": rng.standard_normal(D, dtype=np.float32) * 0.04,
    }
    t0 = time.time()
    out = kernel(**inputs)
    print("kernel done in", time.time() - t0, "s", out.shape, out.dtype)
